# revision 22
# baseline (speedup 1.0000x reference)
"""Trainium2 Bass kernel for nn_DotMatrix.

Math: for each (b, ell, t) the reference computes a complex pairwise dot
matrix O[i,j] = sum_m z[i,m] * w[j,m] where z = rep[b,:,t,:,:] as complex
and w the sign-flipped conjugation partner.  As a real matmul:

  lhsT[k, i]   k = (c,m) stacked: [Zr.T; Zi.T]                 [2m, 256]
  rhs[k, 2j+c'] c'=0: [FZr; -FZi], c'=1: [FZi; FZr]            [2m, 512]
  out = lhsT.T @ rhs  -> [256 i, 512 (j,c)]

with FZr[m',j] = s[m'] * Zr[j, M-1-m'], s[m'] = (-1)^(ell+m').

Precision trick: fp32 matmuls run at 4 cycles/column on the PE, but the
contraction dim here is tiny (2m <= 14), so we decompose each operand
into three bf16 parts (hi/mid/lo, 24 mantissa bits total) and stack the
six significant cross terms along the dead K dimension:

  L = [Ah; Am; Al; Ah; Am; Ah]   R = [Bh; Bh; Bh; Bm; Bm; Bl]

One bf16 matmul (K = 6*2m <= 84) then equals the fp32 product to
~2^-24, at 1 cycle/column — 4x faster than the fp32 path and with fast
(FWL) weight loads.

Sharding: 8 cores = 2 batches x 4 tau-quarters.  Each core owns 32
channels ch = ell*8 + s (t = tq*8 + s), computes the full 256x256
matrix per channel, and writes [32, 2, 128, 512] fp32; channel pairs
share one 1MB contiguous DMA.  Host reassembles [2,256,256,128,2].
"""

import numpy as np
import ml_dtypes

import concourse.bass as bass
import concourse.bacc as bacc
import concourse.mybir as mybir
from concourse.bass_utils import run_bass_kernel_spmd
from concourse.tile import TileContext

B, N, TAU, NELL = 2, 256, 32, 4
NCORES = 8
NCH = 32          # channels per core (4 ell * 8 slots)
F32 = mybir.dt.float32
BF16 = mybir.dt.bfloat16
BFNP = ml_dtypes.bfloat16
KS = [6 * 2 * (2 * ell + 1) for ell in range(NELL)]   # 12, 36, 60, 84
BIW = [512 - 128 * bi for bi in range(4)]             # cols per i-block
BIO = [0, 512, 896, 1152]                             # ot offsets per i-block
OTW = 1280                                            # sum(BIW)

_NC_CACHE = {}


def _build_bass():
    nc = bacc.Bacc()
    # Inputs packed to full 128-partition height for port-parallel DMA:
    # tensor A rows 0:84 = ell3 K-stack, rows 96:108 = ell0; tensor B rows
    # 0:60 = ell2, rows 64:100 = ell1 (matmul base partitions 0/64/96).
    lhs_d = [
        nc.declare_dram_parameter(f"lhs{t}", [128, 8 * 256], BF16, isOutput=False)
        for t in ("A", "B")
    ]
    rhs_d = [
        nc.declare_dram_parameter(f"rhs{t}", [128, 8 * 512], BF16, isOutput=False)
        for t in ("A", "B")
    ]
    # The pairwise matrix is symmetric in (i,j), so each channel only
    # computes i-blocks of 64 against j >= 64*bi (block upper triangle,
    # 62.5% of the full matrix); the host mirrors the rest.  Two channels
    # (a pair) share each matmul's 128 PSUM partitions via column tiling.
    # Layout: [pair, psum_row, (bi-block columns)] — contiguous per pair.
    out = nc.declare_dram_parameter("out", [NCH // 2, 128, OTW], F32, isOutput=True)

    with TileContext(nc) as tc:
        with (
            tc.tile_pool(name="lin", bufs=1) as lin_pool,
            tc.tile_pool(name="rin", bufs=1) as rin_pool,
            tc.tile_pool(name="ps", bufs=8, space="PSUM") as ps_pool,
            tc.tile_pool(name="ot", bufs=5) as ot_pool,
        ):
            lhs_sb = [lin_pool.tile([128, 8 * 256], BF16, tag=f"l{t}", name=f"lhs_sb{t}") for t in range(2)]
            rhs_sb = [rin_pool.tile([128, 8 * 512], BF16, tag=f"r{t}", name=f"rhs_sb{t}") for t in range(2)]
            # ell -> (packed tensor idx, base partition)
            pack = {3: (0, 0), 0: (0, 96), 2: (1, 0), 1: (1, 64)}
            # input loads on the dedicated gpsimd queue (round-robins with
            # output stores at packet granularity); half-tensor chunks so
            # compute starts after the first two
            for c in range(2):
                for t in range(2):
                    nc.scalar.dma_start(
                        out=lhs_sb[t][:, c * 1024 : (c + 1) * 1024],
                        in_=lhs_d[t][:, c * 1024 : (c + 1) * 1024],
                    )
                    nc.scalar.dma_start(
                        out=rhs_sb[t][:, c * 2048 : (c + 1) * 2048],
                        in_=rhs_d[t][:, c * 2048 : (c + 1) * 2048],
                    )
            n_copy = 0
            for c in range(2):                  # slot chunk (4 slots each)
                for e in (0, 3, 2, 1):          # A-tensor ells first
                    K = KS[e]
                    t, bp = pack[e]
                    for u in (2 * c, 2 * c + 1):    # channel pair
                        ot = ot_pool.tile([128, OTW], F32)
                        for bi in range(4):     # i-block of 64 rows
                            W = BIW[bi]
                            ps = ps_pool.tile([128, 512], F32)
                            for c2 in range(2):  # channel within pair
                                sl = u * 2 + c2
                                nc.tensor.matmul(
                                    ps[c2 * 64 : (c2 + 1) * 64, 0:W],
                                    lhs_sb[t][
                                        bp : bp + K,
                                        sl * 256 + bi * 64 : sl * 256 + bi * 64 + 64,
                                    ],
                                    rhs_sb[t][
                                        bp : bp + K, sl * 512 + 128 * bi : (sl + 1) * 512
                                    ],
                                    start=True,
                                    stop=True,
                                    tile_position=(bp, c2 * 64),
                                )
                            dst = ot[:, BIO[bi] : BIO[bi] + W]
                            if n_copy % 2 == 0:
                                nc.scalar.copy(dst, ps[:, 0:W])
                            else:
                                nc.vector.tensor_copy(out=dst, in_=ps[:, 0:W])
                            n_copy += 1
                        nc.sync.dma_start(out=out[e * 4 + u], in_=ot[:])
    nc.compile()
    return nc


def _dec3(x):
    h = x.astype(BFNP)
    r = x - h.astype(np.float32)
    m_ = r.astype(BFNP)
    l = (r - m_.astype(np.float32)).astype(BFNP)
    return h, m_, l


_PACK = {3: (0, 0), 0: (0, 96), 2: (1, 0), 1: (1, 64)}


def _host_prep(reps, cid):
    """Build per-core bf16 K-stacked lhs/rhs tensors (partition-packed)."""
    b, tq = cid // 4, cid % 4
    im = {
        "lhsA": np.zeros((128, 8 * 256), BFNP),
        "lhsB": np.zeros((128, 8 * 256), BFNP),
        "rhsA": np.zeros((128, 8 * 512), BFNP),
        "rhsB": np.zeros((128, 8 * 512), BFNP),
    }
    for ell in range(NELL):
        rep = reps[ell]
        m = 2 * ell + 1
        s_vec = ((-1.0) ** (ell + np.arange(m))).astype(np.float32)
        t, bp = _PACK[ell]
        LHS = im["lhsA" if t == 0 else "lhsB"]
        RHS = im["rhsA" if t == 0 else "rhsB"]
        for sidx in range(8):
            t = tq * 8 + sidx
            Z = rep[b, :, t]                      # [256, m, 2]
            Zr, Zi = Z[..., 0], Z[..., 1]         # [256, m]
            lhsT = np.concatenate([Zr.T, Zi.T], axis=0)      # [2m, 256]
            FZr = s_vec[:, None] * Zr[:, ::-1].T             # [m, 256]
            FZi = s_vec[:, None] * Zi[:, ::-1].T
            R = np.empty((2 * m, 256, 2), np.float32)
            R[0:m, :, 0] = FZr
            R[m:, :, 0] = -FZi
            R[0:m, :, 1] = FZi
            R[m:, :, 1] = FZr
            rhs = R.reshape(2 * m, 512)
            Ah, Am, Al = _dec3(lhsT)
            Bh, Bm, Bl = _dec3(rhs)
            LHS[bp : bp + KS[ell], sidx * 256 : (sidx + 1) * 256] = np.concatenate(
                [Ah, Am, Al, Ah, Am, Ah], axis=0
            )
            RHS[bp : bp + KS[ell], sidx * 512 : (sidx + 1) * 512] = np.concatenate(
                [Bh, Bh, Bh, Bm, Bm, Bl], axis=0
            )
    return im


def _run(in_maps, **kw):
    if "nc" not in _NC_CACHE:
        _NC_CACHE["nc"] = _build_bass()
    return run_bass_kernel_spmd(_NC_CACHE["nc"], in_maps, list(range(NCORES)), **kw)


def kernel(rep0, rep1, rep2, rep3, _bass_kw=None):
    reps = [np.ascontiguousarray(np.asarray(r, dtype=np.float32)) for r in (rep0, rep1, rep2, rep3)]
    in_maps = [_host_prep(reps, cid) for cid in range(NCORES)]
    res = _run(in_maps, **(_bass_kw or {}))
    out = np.empty((B, N, N, NELL * TAU, 2), np.float32)
    for cid in range(NCORES):
        b, tq = cid // 4, cid % 4
        arr = res.results[cid]["out"]          # [16, 128, OTW]
        o = np.empty((NCH, 256, 256, 2), np.float32)
        for bi in range(4):
            nj = 256 - 64 * bi
            blk = arr[:, :, BIO[bi] : BIO[bi] + BIW[bi]].reshape(
                NCH // 2, 2, 64, nj, 2
            )
            # blk[pair, c2, i_in_block, j - 64*bi, c]
            o[0::2, 64 * bi : 64 * bi + 64, 64 * bi :, :] = blk[:, 0]
            o[1::2, 64 * bi : 64 * bi + 64, 64 * bi :, :] = blk[:, 1]
        for bi in range(1, 4):                  # mirror lower block triangle
            r = slice(64 * bi, 64 * bi + 64)
            o[:, r, : 64 * bi, :] = o[:, : 64 * bi, r, :].transpose(0, 2, 1, 3)
        for ell in range(NELL):
            lo = ell * TAU + tq * 8
            out[b, :, :, lo : lo + 8, :] = o[ell * 8 : (ell + 1) * 8].transpose(
                1, 2, 0, 3
            )
    kernel.last_result = res
    return out


# revision 23
# speedup vs baseline: 1.0583x; 1.0583x over previous
"""Trainium2 Bass kernel for nn_DotMatrix.

Math: for each (b, ell, t) the reference computes a complex pairwise dot
matrix O[i,j] = sum_m z[i,m] * w[j,m] where z = rep[b,:,t,:,:] as complex
and w the sign-flipped conjugation partner.  As a real matmul:

  lhsT[k, i]   k = (c,m) stacked: [Zr.T; Zi.T]                 [2m, 256]
  rhs[k, 2j+c'] c'=0: [FZr; -FZi], c'=1: [FZi; FZr]            [2m, 512]
  out = lhsT.T @ rhs  -> [256 i, 512 (j,c)]

with FZr[m',j] = s[m'] * Zr[j, M-1-m'], s[m'] = (-1)^(ell+m').

Precision trick: fp32 matmuls run at 4 cycles/column on the PE, but the
contraction dim here is tiny (2m <= 14), so we decompose each operand
into three bf16 parts (hi/mid/lo, 24 mantissa bits total) and stack the
six significant cross terms along the dead K dimension:

  L = [Ah; Am; Al; Ah; Am; Ah]   R = [Bh; Bh; Bh; Bm; Bm; Bl]

One bf16 matmul (K = 6*2m <= 84) then equals the fp32 product to
~2^-24, at 1 cycle/column — 4x faster than the fp32 path and with fast
(FWL) weight loads.

Sharding: 8 cores = 2 batches x 4 tau-quarters.  Each core owns 32
channels ch = ell*8 + s (t = tq*8 + s), computes the full 256x256
matrix per channel, and writes [32, 2, 128, 512] fp32; channel pairs
share one 1MB contiguous DMA.  Host reassembles [2,256,256,128,2].
"""

import numpy as np
import ml_dtypes

import concourse.bass as bass
import concourse.bacc as bacc
import concourse.mybir as mybir
from concourse.bass_utils import run_bass_kernel_spmd
from concourse.tile import TileContext

B, N, TAU, NELL = 2, 256, 32, 4
NCORES = 8
NCH = 32          # channels per core (4 ell * 8 slots)
F32 = mybir.dt.float32
BF16 = mybir.dt.bfloat16
BFNP = ml_dtypes.bfloat16
KS = [6 * 2 * (2 * ell + 1) for ell in range(NELL)]   # 12, 36, 60, 84
BIW = [512 - 128 * bi for bi in range(4)]             # cols per i-block
BIO = [0, 512, 896, 1152]                             # ot offsets per i-block
OTW = 1280                                            # sum(BIW)

_NC_CACHE = {}


def _build_bass():
    nc = bacc.Bacc()
    # Inputs packed to full 128-partition height for port-parallel DMA:
    # tensor A rows 0:84 = ell3 K-stack, rows 96:108 = ell0; tensor B rows
    # 0:60 = ell2, rows 64:100 = ell1 (matmul base partitions 0/64/96).
    lhs_d = [
        nc.declare_dram_parameter(f"lhs{t}", [128, 8 * 256], BF16, isOutput=False)
        for t in ("A", "B")
    ]
    rhs_d = [
        nc.declare_dram_parameter(f"rhs{t}", [128, 8 * 512], BF16, isOutput=False)
        for t in ("A", "B")
    ]
    # The pairwise matrix is symmetric in (i,j), so each channel only
    # computes i-blocks of 64 against j >= 64*bi (block upper triangle,
    # 62.5% of the full matrix); the host mirrors the rest.  Two channels
    # (a pair) share each matmul's 128 PSUM partitions via column tiling.
    # Layout: [pair, psum_row, (bi-block columns)] — contiguous per pair.
    out = nc.declare_dram_parameter("out", [NCH // 2, 128, OTW], F32, isOutput=True)

    with TileContext(nc) as tc:
        with (
            tc.tile_pool(name="lin", bufs=1) as lin_pool,
            tc.tile_pool(name="rin", bufs=1) as rin_pool,
            tc.tile_pool(name="ps", bufs=8, space="PSUM") as ps_pool,
            tc.tile_pool(name="ot", bufs=5) as ot_pool,
        ):
            lhs_sb = [lin_pool.tile([128, 8 * 256], BF16, tag=f"l{t}", name=f"lhs_sb{t}") for t in range(2)]
            rhs_sb = [rin_pool.tile([128, 8 * 512], BF16, tag=f"r{t}", name=f"rhs_sb{t}") for t in range(2)]
            # ell -> (packed tensor idx, base partition)
            pack = {3: (0, 0), 0: (0, 96), 2: (1, 0), 1: (1, 64)}
            # First input chunk rides the sync HWDGE ring (fast first byte,
            # ahead of any output in its FIFO); the rest stream on the
            # dedicated gpsimd queue, round-robining with output stores.
            for c in range(2):
                for t in range(2):
                    eng = nc.sync if (c == 0 and t == 0) else nc.gpsimd
                    eng.dma_start(
                        out=lhs_sb[t][:, c * 1024 : (c + 1) * 1024],
                        in_=lhs_d[t][:, c * 1024 : (c + 1) * 1024],
                    )
                    eng.dma_start(
                        out=rhs_sb[t][:, c * 2048 : (c + 1) * 2048],
                        in_=rhs_d[t][:, c * 2048 : (c + 1) * 2048],
                    )
            n_copy = 0
            for c in range(2):                  # slot chunk (4 slots each)
                for e in (0, 3, 2, 1):          # A-tensor ells first
                    K = KS[e]
                    t, bp = pack[e]
                    for u in (2 * c, 2 * c + 1):    # channel pair
                        ot = ot_pool.tile([128, OTW], F32)
                        for bi in range(4):     # i-block of 64 rows
                            W = BIW[bi]
                            ps = ps_pool.tile([128, 512], F32)
                            for c2 in range(2):  # channel within pair
                                sl = u * 2 + c2
                                nc.tensor.matmul(
                                    ps[c2 * 64 : (c2 + 1) * 64, 0:W],
                                    lhs_sb[t][
                                        bp : bp + K,
                                        sl * 256 + bi * 64 : sl * 256 + bi * 64 + 64,
                                    ],
                                    rhs_sb[t][
                                        bp : bp + K, sl * 512 + 128 * bi : (sl + 1) * 512
                                    ],
                                    start=True,
                                    stop=True,
                                    tile_position=(bp, c2 * 64),
                                )
                            dst = ot[:, BIO[bi] : BIO[bi] + W]
                            if n_copy % 2 == 0:
                                nc.scalar.copy(dst, ps[:, 0:W])
                            else:
                                nc.vector.tensor_copy(out=dst, in_=ps[:, 0:W])
                            n_copy += 1
                        nc.sync.dma_start(out=out[e * 4 + u], in_=ot[:])
    nc.compile()
    return nc


def _dec3(x):
    h = x.astype(BFNP)
    r = x - h.astype(np.float32)
    m_ = r.astype(BFNP)
    l = (r - m_.astype(np.float32)).astype(BFNP)
    return h, m_, l


_PACK = {3: (0, 0), 0: (0, 96), 2: (1, 0), 1: (1, 64)}


def _host_prep(reps, cid):
    """Build per-core bf16 K-stacked lhs/rhs tensors (partition-packed)."""
    b, tq = cid // 4, cid % 4
    im = {
        "lhsA": np.zeros((128, 8 * 256), BFNP),
        "lhsB": np.zeros((128, 8 * 256), BFNP),
        "rhsA": np.zeros((128, 8 * 512), BFNP),
        "rhsB": np.zeros((128, 8 * 512), BFNP),
    }
    for ell in range(NELL):
        rep = reps[ell]
        m = 2 * ell + 1
        s_vec = ((-1.0) ** (ell + np.arange(m))).astype(np.float32)
        t, bp = _PACK[ell]
        LHS = im["lhsA" if t == 0 else "lhsB"]
        RHS = im["rhsA" if t == 0 else "rhsB"]
        for sidx in range(8):
            t = tq * 8 + sidx
            Z = rep[b, :, t]                      # [256, m, 2]
            Zr, Zi = Z[..., 0], Z[..., 1]         # [256, m]
            lhsT = np.concatenate([Zr.T, Zi.T], axis=0)      # [2m, 256]
            FZr = s_vec[:, None] * Zr[:, ::-1].T             # [m, 256]
            FZi = s_vec[:, None] * Zi[:, ::-1].T
            R = np.empty((2 * m, 256, 2), np.float32)
            R[0:m, :, 0] = FZr
            R[m:, :, 0] = -FZi
            R[0:m, :, 1] = FZi
            R[m:, :, 1] = FZr
            rhs = R.reshape(2 * m, 512)
            Ah, Am, Al = _dec3(lhsT)
            Bh, Bm, Bl = _dec3(rhs)
            LHS[bp : bp + KS[ell], sidx * 256 : (sidx + 1) * 256] = np.concatenate(
                [Ah, Am, Al, Ah, Am, Ah], axis=0
            )
            RHS[bp : bp + KS[ell], sidx * 512 : (sidx + 1) * 512] = np.concatenate(
                [Bh, Bh, Bh, Bm, Bm, Bl], axis=0
            )
    return im


def _run(in_maps, **kw):
    if "nc" not in _NC_CACHE:
        _NC_CACHE["nc"] = _build_bass()
    return run_bass_kernel_spmd(_NC_CACHE["nc"], in_maps, list(range(NCORES)), **kw)


def kernel(rep0, rep1, rep2, rep3, _bass_kw=None):
    reps = [np.ascontiguousarray(np.asarray(r, dtype=np.float32)) for r in (rep0, rep1, rep2, rep3)]
    in_maps = [_host_prep(reps, cid) for cid in range(NCORES)]
    res = _run(in_maps, **(_bass_kw or {}))
    out = np.empty((B, N, N, NELL * TAU, 2), np.float32)
    for cid in range(NCORES):
        b, tq = cid // 4, cid % 4
        arr = res.results[cid]["out"]          # [16, 128, OTW]
        o = np.empty((NCH, 256, 256, 2), np.float32)
        for bi in range(4):
            nj = 256 - 64 * bi
            blk = arr[:, :, BIO[bi] : BIO[bi] + BIW[bi]].reshape(
                NCH // 2, 2, 64, nj, 2
            )
            # blk[pair, c2, i_in_block, j - 64*bi, c]
            o[0::2, 64 * bi : 64 * bi + 64, 64 * bi :, :] = blk[:, 0]
            o[1::2, 64 * bi : 64 * bi + 64, 64 * bi :, :] = blk[:, 1]
        for bi in range(1, 4):                  # mirror lower block triangle
            r = slice(64 * bi, 64 * bi + 64)
            o[:, r, : 64 * bi, :] = o[:, : 64 * bi, r, :].transpose(0, 2, 1, 3)
        for ell in range(NELL):
            lo = ell * TAU + tq * 8
            out[b, :, :, lo : lo + 8, :] = o[ell * 8 : (ell + 1) * 8].transpose(
                1, 2, 0, 3
            )
    kernel.last_result = res
    return out
